# revision 9
# baseline (speedup 1.0000x reference)
"""Causal segment-masked depthwise conv (K=5) + pointwise conv, 8-core SPMD.

Strategy (v2.2, bf16):
  Host: pack each batch row's segments into a global stream with 4 zeros
  before each segment (plain causal conv on the stream == per-segment
  left-zero-padded conv), split the stream evenly across 8 cores with a
  4-element halo, pre-transpose to [C, stream] and cast to bf16.
  Biases fold out of the device: out = Wpw.conv + (Wpw@b_dw + b_pw); the
  constant rides the ACT out-copies' per-partition bias operand.
  Device per core (stream width 4160 = 4 superblocks of 1024 + 64 tail):
    dw conv:
      PE   : chunks 0,1 (diag-stationary bf16 matmuls, tap-major over the
             two 512-blocks of each superblock so each LDWEIGHTS serves
             two matmuls; PSUM [128,1024] f32, one 1024-wide ACT copy ->
             bf16), full tail.
      DVE  : chunks 2,3 as tensor_scalar products (bf16 fast mode) +
             tensor_tensor adds (bf16 2x mode); odd-shift taps read a
             host-packed 1-element-shifted dup slab so every DVE stream
             stays 4B-aligned.
    pw: dch-pair major per superblock: for each output-chunk pair, 16
        bf16 matmuls (4 j x 2 dch x 2 blocks) with j=2,3 (DVE inputs)
        last, each weight load serving two matmuls into the two halves
        of a [128,1024] PSUM tile; ACT copies PSUM -> bf16 out tile
        adding the folded bias; one DMA store per superblock.
  Host transposes back during gather and applies a sparse general-case
  correction for exotic segment overlap patterns (empty for contiguous
  partitions).
"""

import sys

sys.path.insert(0, "/opt/trn_rl_repo")

import numpy as np
import ml_dtypes

BF16 = ml_dtypes.bfloat16

B, L, C, K, S = 8, 4096, 512, 5, 8
NCORES = 8
CCH = C // 128          # 4 channel chunks
NSB = 4                 # superblocks of 1024
SBW = NSB * 1024        # 4096
TAILW = 64              # tail block width
TOTW = SBW + TAILW      # 4160 per-core processed stream width
XSW = 1040              # packed slab piece width (1024 + 4 halo + pad)
XQW = 72                # tail slab width (64 + 4 halo + pad)

_cached = {}


def _build_nc():
    import concourse.mybir as mybir
    from concourse import bacc
    from concourse.tile import TileContext

    f32 = mybir.dt.float32
    bf16 = mybir.dt.bfloat16

    nc = bacc.Bacc(num_swdge_queues=2)
    # planes: 0,1 = chunks 0,1 ; 2,3 = chunk2, dup2 ; 4,5 = chunk3, dup3
    xs_d = nc.declare_dram_parameter("xs", [NSB, 128, 6, XSW], bf16, isOutput=False)
    xq_d = nc.declare_dram_parameter("xq", [128, CCH, XQW], bf16, isOutput=False)
    # cst: [0:20]=wdiag f32, [20:24]=bout f32
    cst_d = nc.declare_dram_parameter("cst", [128, CCH * K + CCH], f32, isOutput=False)
    diag_d = nc.declare_dram_parameter("diag", [128, CCH * K, 128], bf16, isOutput=False)
    wpwt_d = nc.declare_dram_parameter("wpwt", [128, CCH, CCH, 128], bf16, isOutput=False)
    out_d = nc.declare_dram_parameter("out", [128, CCH, TOTW], bf16, isOutput=True)

    with TileContext(nc) as tc:
        with (
            tc.tile_pool(name="consts", bufs=1) as cpool,
            tc.tile_pool(name="xc0", bufs=2) as x0_pool,
            tc.tile_pool(name="xc1", bufs=2) as x1_pool,
            tc.tile_pool(name="xc2", bufs=2) as x2_pool,
            tc.tile_pool(name="xc3", bufs=2) as x3_pool,
            tc.tile_pool(name="acc2", bufs=2) as a2_pool,
            tc.tile_pool(name="acc3", bufs=2) as a3_pool,
            tc.tile_pool(name="tprod", bufs=2) as tp_pool,
            tc.tile_pool(name="dwt", bufs=4) as dwt_pool,
            tc.tile_pool(name="outsb", bufs=2) as ob_pool,
            tc.tile_pool(name="dps", bufs=2, space="PSUM") as dps,
            tc.tile_pool(name="pwps", bufs=2, space="PSUM") as pwps,
        ):
            # ---- consts (sync ring first) ----
            cst = cpool.tile([128, CCH * K + CCH], f32)
            nc.sync.dma_start(out=cst[:], in_=cst_d[:])
            wdiag = cst[:, 0 : CCH * K]
            bout = cst[:, CCH * K : CCH * K + CCH]
            xq = cpool.tile([128, CCH, XQW], bf16)
            nc.sync.dma_start(out=xq[:], in_=xq_d[:])

            # warm-up fodder: zero tile, no DMA dependency (Pool memset)
            warmz = cpool.tile([128, 512], bf16)
            nc.gpsimd.memset(warmz[:], 0.0)

            xt = {}

            def load_sync(pool, sb, plane, tag):
                t = pool.tile([128, XSW], bf16, tag=tag, name=f"{tag}_{sb}")
                nc.sync.dma_start(out=t[:], in_=xs_d[sb, :, plane, :])
                xt[(sb, tag)] = t

            def load_pair(poolx, sb, plane, tagx):
                t = poolx.tile([128, 2, XSW], bf16, tag=tagx, name=f"{tagx}_{sb}")
                nc.gpsimd.dma_start(out=t[:], in_=xs_d[sb, :, plane : plane + 2, :])
                xt[(sb, tagx)] = t

            # sync ring: consts + PE chunks (chunk0/1 of sb0 ahead of wpwt)
            load_sync(x0_pool, 0, 0, "x0")
            load_sync(x1_pool, 0, 1, "x1")
            wpwt = cpool.tile([128, CCH, CCH, 128], bf16)
            nc.sync.dma_start(out=wpwt[:], in_=wpwt_d[:])
            for sb in range(1, NSB):
                load_sync(x0_pool, sb, 0, "x0")
                load_sync(x1_pool, sb, 1, "x1")

            # SWDGE queues: q~a gets chunk2+dup2 pieces, q~b gets diag then
            # chunk3+dup3 (chunk3 isn't consumed until ~7us in)
            diag = cpool.tile([128, CCH * K, 128], bf16)
            load_pair(x2_pool, 0, 2, "x2")
            nc.gpsimd.dma_start(out=diag[:], in_=diag_d[:])
            load_pair(x3_pool, 0, 4, "x3")
            for sb in range(1, NSB):
                load_pair(x2_pool, sb, 2, "x2")
                load_pair(x3_pool, sb, 4, "x3")

            # PE warm-up: lift the HAM clock gate while DMAs land
            for wi in range(6):
                wps = pwps.tile([128, 512], f32, tag="pwps", name=f"warm{wi}")
                nc.tensor.matmul(
                    wps[:], lhsT=warmz[:, 0:128], rhs=warmz[:], start=True, stop=True
                )

            # ---- DVE dw conv for one chunk over one superblock ----
            def dve_chunk(sb, c, pool):
                AB = xt[(sb, f"x{c}")]
                acc = pool.tile([128, 1024], bf16, tag=f"a{c}", name=f"a{c}_{sb}")
                nc.vector.tensor_scalar_mul(
                    acc[:], AB[:, 0, 0:1024], wdiag[:, c * K : c * K + 1]
                )
                for k in range(1, K):
                    tp = tp_pool.tile(
                        [128, 1024], bf16, tag="tp", name=f"tp{c}_{sb}_{k}"
                    )
                    src = (
                        AB[:, 1, k - 1 : k - 1 + 1024]
                        if k % 2
                        else AB[:, 0, k : k + 1024]
                    )
                    nc.vector.tensor_scalar_mul(
                        tp[:], src, wdiag[:, c * K + k : c * K + k + 1]
                    )
                    nc.vector.tensor_add(acc[:], acc[:], tp[:])
                return acc

            # ---- PE dw conv for one chunk over one superblock (tap-major) ----
            def pe_conv(sb, c):
                ps = dps.tile([128, 1024], f32, tag="dps", name=f"ps{c}_{sb}")
                x = xt[(sb, f"x{c}")]
                for k in range(K):
                    for bb in range(2):
                        o = bb * 512
                        nc.tensor.matmul(
                            ps[:, o : o + 512],
                            lhsT=diag[:, c * K + k, :],
                            rhs=x[:, o + k : o + k + 512],
                            start=(k == 0),
                            stop=(k == K - 1),
                        )
                dt_ = dwt_pool.tile([128, 1024], bf16, tag="dwt", name=f"dwt{c}_{sb}")
                nc.scalar.copy(dt_[:], ps[:])
                return dt_

            # ---- pointwise for one superblock, one dch pair ----
            def pointwise_pair(sb, pair, rhs_of, ob):
                pos = {}
                for dch in pair:
                    pos[dch] = pwps.tile(
                        [128, 1024], f32, tag="pwps", name=f"po{dch}_{sb}"
                    )
                for jj in range(CCH):  # j=2,3 (DVE inputs) last
                    for dch in pair:
                        for bb in range(2):
                            o = bb * 512
                            nc.tensor.matmul(
                                pos[dch][:, o : o + 512],
                                lhsT=wpwt[:, jj, dch, :],
                                rhs=rhs_of[jj][:, o : o + 512],
                                start=(jj == 0),
                                stop=(jj == CCH - 1),
                            )
                for dch in pair:
                    nc.scalar.add(
                        ob[:, dch, :], pos[dch][:], bout[:, dch : dch + 1]
                    )

            # ---- tail block (cols 4096..4159), all 4 chunks on PE ----
            def tail_block():
                dwq = []
                for c in range(CCH):
                    ps = dps.tile([128, TAILW], f32, tag="dps", name=f"psq{c}")
                    for k in range(K):
                        nc.tensor.matmul(
                            ps[:],
                            lhsT=diag[:, c * K + k, :],
                            rhs=xq[:, c, k : k + TAILW],
                            start=(k == 0),
                            stop=(k == K - 1),
                        )
                    dt_ = dwt_pool.tile(
                        [128, TAILW], bf16, tag="dwt", name=f"dwq{c}"
                    )
                    nc.scalar.copy(dt_[:], ps[:])
                    dwq.append(dt_)
                pos = [
                    pwps.tile([128, TAILW], f32, tag="pwps", name=f"poq{dch}")
                    for dch in range(CCH)
                ]
                for j in range(CCH):
                    for dch in range(CCH):
                        nc.tensor.matmul(
                            pos[dch][:],
                            lhsT=wpwt[:, j, dch, :],
                            rhs=dwq[j][:],
                            start=(j == 0),
                            stop=(j == CCH - 1),
                        )
                ob = ob_pool.tile([128, CCH, TAILW], bf16, tag="obq", name="ob_q")
                for dch in range(CCH):
                    nc.scalar.add(ob[:, dch, :], pos[dch][:], bout[:, dch : dch + 1])
                nc.sync.dma_start(out=out_d[:, :, SBW : SBW + TAILW], in_=ob[:])

            # ---- main pipeline ----
            tail_block()
            for sb in range(NSB):
                a2 = dve_chunk(sb, 2, a2_pool)
                dwt0 = pe_conv(sb, 0)
                dwt1 = pe_conv(sb, 1)
                a3 = dve_chunk(sb, 3, a3_pool)
                rhs_of = {0: dwt0, 1: dwt1, 2: a2, 3: a3}
                ob = ob_pool.tile(
                    [128, CCH, 1024], bf16, tag="ob", name=f"ob_{sb}"
                )
                pointwise_pair(sb, (0, 1), rhs_of, ob)
                pointwise_pair(sb, (2, 3), rhs_of, ob)
                off = sb * 1024
                st = (nc.sync, nc.scalar, nc.gpsimd, nc.gpsimd)[sb]
                st.dma_start(out=out_d[:, :, off : off + 1024], in_=ob[:])

    nc.finalize()
    return nc


def _get_nc():
    if "nc" not in _cached:
        _cached["nc"] = _build_nc()
    return _cached["nc"]


def _analyze(segment_boundaries):
    starts = segment_boundaries[..., 0].astype(np.int64)  # [B,S]
    ends = segment_boundaries[..., 1].astype(np.int64)
    pos = np.arange(L)
    in_seg = (pos[None, None, :] >= starts[..., None]) & (
        pos[None, None, :] < ends[..., None]
    )  # [B,S,L]
    covered = in_seg.any(axis=1)
    seg_id = np.where(covered, in_seg.argmax(axis=1), -1)  # [B,L]
    return covered, seg_id


def kernel(x, segment_boundaries, w_dw, b_dw, w_pw, b_pw):
    from concourse.bass_utils import run_bass_kernel_spmd

    x = np.asarray(x, dtype=np.float32)
    sb = np.asarray(segment_boundaries)
    w_dw = np.asarray(w_dw, dtype=np.float32)
    b_dw = np.asarray(b_dw, dtype=np.float32)
    w_pw = np.asarray(w_pw, dtype=np.float32)
    b_pw = np.asarray(b_pw, dtype=np.float32)

    covered, seg_id = _analyze(sb)

    # ---- run decomposition + stream build ----
    pieces = []
    src_b_parts = []
    src_l_parts = []
    run_start_of = np.full((B, L), -1, np.int64)
    for b in range(B):
        sid = seg_id[b]
        change = np.nonzero(np.diff(sid) != 0)[0] + 1
        bounds = np.concatenate([[0], change, [L]])
        for s, e in zip(bounds[:-1], bounds[1:]):
            if sid[s] < 0:
                continue
            run_start_of[b, s:e] = s
            pieces.append(np.zeros((4, C), np.float32))
            src_b_parts.append(np.full(4, -1, np.int64))
            src_l_parts.append(np.full(4, -1, np.int64))
            pieces.append(x[b, s:e])
            src_b_parts.append(np.full(e - s, b, np.int64))
            src_l_parts.append(np.arange(s, e, dtype=np.int64))
    if pieces:
        stream = np.concatenate(pieces, axis=0)
        src_b = np.concatenate(src_b_parts)
        src_l = np.concatenate(src_l_parts)
    else:
        stream = np.zeros((0, C), np.float32)
        src_b = np.zeros(0, np.int64)
        src_l = np.zeros(0, np.int64)
    T = stream.shape[0]
    Q = -(-T // NCORES) if T else 1
    assert Q <= TOTW, f"stream quota {Q} too large"

    # ---- shared per-core inputs ----
    wdiag = np.ascontiguousarray(
        w_dw.reshape(CCH, 128, K).transpose(1, 0, 2).reshape(128, CCH * K)
    ).astype(np.float32)
    bias_out = w_pw @ b_dw + b_pw
    boutr = np.ascontiguousarray(bias_out.reshape(CCH, 128).T).astype(np.float32)
    cst = np.concatenate([wdiag, boutr], axis=1)
    diag = np.zeros((128, CCH * K, 128), BF16)
    idx = np.arange(128)
    diag[idx, :, idx] = wdiag.astype(BF16)[idx, :]
    wpwt = np.ascontiguousarray(
        w_pw.reshape(CCH, 128, CCH, 128).transpose(3, 2, 0, 1)
    ).astype(BF16)

    SLAB_W = 4 + TOTW + 16
    in_maps = []
    spans = []
    for i in range(NCORES):
        lo, hi = i * Q, min((i + 1) * Q, T)
        lo = min(lo, T)
        spans.append((lo, hi))
        buf = np.zeros((SLAB_W, C), np.float32)
        if hi > lo:
            hlo = max(0, lo - 4)
            buf[4 - (lo - hlo) : 4 + (hi - lo)] = stream[hlo:hi]
        slabT = np.ascontiguousarray(buf.T).astype(BF16)  # [C, SLAB_W]
        slabT = slabT.reshape(CCH, 128, SLAB_W)
        xs = np.zeros((NSB, 128, 6, XSW), BF16)
        for sbi in range(NSB):
            off = sbi * 1024
            piece = slabT[:, :, off : off + 1032]            # [CCH,128,1032]
            dpiece = slabT[:, :, off + 1 : off + 1033]
            xs[sbi, :, 0, :1032] = piece[0]
            xs[sbi, :, 1, :1032] = piece[1]
            xs[sbi, :, 2, :1032] = piece[2]
            xs[sbi, :, 3, :1032] = dpiece[2]
            xs[sbi, :, 4, :1032] = piece[3]
            xs[sbi, :, 5, :1032] = dpiece[3]
        xq = np.zeros((128, CCH, XQW), BF16)
        xq[:, :, : TAILW + 4] = slabT[:, :, SBW : SBW + TAILW + 4].transpose(1, 0, 2)
        in_maps.append({"xs": xs, "xq": xq, "cst": cst, "diag": diag, "wpwt": wpwt})

    nc = _get_nc()
    res = run_bass_kernel_spmd(nc, in_maps, list(range(NCORES)))

    # ---- gather (device out is [128, CCH, TOTW] bf16) ----
    so_out = np.zeros((T, C), np.float32)
    for i, (lo, hi) in enumerate(spans):
        if hi > lo:
            # [p, ch, t] -> [t, ch*128+p]
            full = (
                np.asarray(res.results[i]["out"])
                .astype(np.float32)
                .transpose(2, 1, 0)
                .reshape(TOTW, C)
            )
            so_out[lo:hi] = full[: hi - lo]
    out = np.zeros((B, L, C), np.float32)
    mask = src_l >= 0
    out[src_b[mask], src_l[mask]] = so_out[mask]

    # ---- general-case sparse correction (pairwise mask vs run mask) ----
    need = []
    for d in range(1, K):
        m_ref = np.zeros((B, L), bool)
        m_ref[:, d:] = covered[:, d:] & (seg_id[:, d:] == seg_id[:, :-d])
        m_run = covered & (np.arange(L)[None, :] - run_start_of >= d)
        diff = m_ref.astype(np.int8) - m_run.astype(np.int8)
        if np.any(diff):
            bs, ls = np.nonzero(diff)
            need.append((d, bs, ls, diff[bs, ls].astype(np.float32)))
    if need:
        for d, bs, ls, sgn in need:
            xv_ = x[bs, ls - d, :]
            delta_dw = xv_ * w_dw[None, :, K - 1 - d] * sgn[:, None]
            out[bs, ls, :] += delta_dw @ w_pw.T

    return out
